# revision 1
# baseline (speedup 1.0000x reference)
"""Trainium2 Bass kernel for nn_DenseEdgeModel (gnn_message_passing).

Reference computation (all 1x1 convs == per-pixel matmuls over channels):
    h    = MLP3(x)                    # 3x (c->c) with ReLU between    [B,C,H,W]
    flat = h as [B*H*W, C]
    xp   = flat[primary_idx]          # [B,PK,C] -> [B,C,PK]
    xc   = flat[compare_idx]          # [B,CK,C] -> [B,C,CK]
    xx   = (xp[..,:,None]-xc[..,None,:])**2          # [B,C,PK,CK]
    g    = relu(W1@xx+b1); g = relu(W2@g+b2)         # over C
    out  = W3@g + b3                  # [B,2,PK,CK]

Sharding (8 cores): data-parallel over batch (4 cores per batch), and the
PK axis split 4-ways within each batch -> each core owns 64 primary
indices of one batch and all 256 compare indices of that batch.

Because the pre-MLP is per-pixel, gather commutes with it:
MLP(x)[idx] == MLP(x[idx]). The host therefore only *slices* (gathers
rows of x for each core's indices and transposes to channel-major).

Device kernel per core:
  stage 1: pre-MLP on the 320 gathered pixel columns [c,320] in fp32r
           (drains split ACT/DVE).  The last layer's weights/bias are
           host-folded by sqrt(SXX) so its bf16 output F is pre-scaled
           for fp8 xx quantization.
  stage 2: per pair of primary columns p ("group"):
    xx   [128,2,512] fp8: channel-chunk 0 via fused ACT Square
           (bias=-xp per primary, FD=256 x2); chunk 1 via DVE
           tensor_scalar subtract (bf16, hits the 4x DVE mode) into a
           bf16 strip, then squared fp8 by one GPSIMD + one DVE
           tensor_tensor (exactly one GPSIMD op per group -- more
           degrades the pipeline).
    l1/l2: ONE fp8e4 DoubleRow matmul per 128-out-chunk (K=256 in a
           single pass at ~2x rate, HW-measured ~230ns for K=256xN=512);
           PSUM holds scale-folded pre-activations; drains
           relu(psum + scaled bias) split ACT/DVE write fp8 (l1) / bf16
           (l2) tiles directly (scales engineered host-side so no
           per-drain rescale op is needed).
    l3   (c->2): bf16 quad-packed into one PSUM bank via tile_position
           col-tiling (bias pre-seeded by a K=1 matmul); one ACT copy +
           4 DMAs drain the quad.  w3 is host-folded by 1/(S1*A2).

Quantization scheme (validated vs reference: rel_err ~1.2e-2 < 2e-2):
  F' = sqrt(SXX)*F;  xx_q = e4m3(SXX*xx);  W1q = e4m3((S1/SXX)*W1)
  psum1 = S1*Z1;  h1_q = e4m3(relu(psum1 + S1*b1)) = e4m3(S1*h1)
  W2q = e4m3(A2*W2);  psum2 = S1*A2*Z2;  h2 = bf16(relu(psum2+S1*A2*b2))
  out = (w3/(S1*A2))@h2 + b3
(e4m3 here is TRN FP8_EXP4: max normal +-240, so scales keep peaks
well under 240.)
"""

import os

import numpy as np

import concourse.bass as bass
import concourse.tile as tile
from concourse import bacc, mybir
from concourse.bass_utils import run_bass_kernel_spmd

# Problem constants (hardcoded per harness contract)
B, C, H, W = 2, 256, 32, 32
PK, CK = 256, 256
N_CORES = 8
CORES_PER_BATCH = N_CORES // B          # 4
P_SHARD = PK // CORES_PER_BATCH         # 64 primary indices per core
NJ = P_SHARD + CK                       # 320 gathered pixel columns per core
PAIR = 2                                # primary columns per stage-2 group
NF = PAIR * CK                          # 512 = stage-2 matmul free dim
QUAD = 4                                # groups sharing one l3 PSUM bank
F32 = mybir.dt.float32
BF16 = mybir.dt.bfloat16
FP8 = mybir.dt.float8e4
MM_DT = mybir.dt.float32r               # stage-1 full-rate fp32 streaming
DR = mybir.MatmulPerfMode.DoubleRow
VARIANT = os.environ.get("KERNEL_VARIANT", "")
AF = mybir.ActivationFunctionType
OP = mybir.AluOpType

# fp8 scale scheme (host-folded; see module docstring)
SXX = 128.0
S1 = 1024.0
A2 = 2048.0


def _build_nc(reps=1):
    nc = bacc.Bacc("TRN2", target_bir_lowering=False, debug=False)

    xgT = nc.dram_tensor("xgT", [C, NJ], F32, kind="ExternalInput")
    pre_wT = nc.dram_tensor("pre_wT", [3, C, C], F32, kind="ExternalInput")
    pre_b = nc.dram_tensor("pre_b", [3, C], F32, kind="ExternalInput")
    # DoubleRow-interleaved fp8 post weights: [oc_chunk, k, k_chunk, m]
    w1q_d = nc.dram_tensor("w1q", [2, 128, 2, 128], FP8, kind="ExternalInput")
    w2q_d = nc.dram_tensor("w2q", [2, 128, 2, 128], FP8, kind="ExternalInput")
    b1s_d = nc.dram_tensor("b1s", [2, 128], F32, kind="ExternalInput")
    b2s_d = nc.dram_tensor("b2s", [2, 128], F32, kind="ExternalInput")
    w3T = nc.dram_tensor("w3T", [C, 2], BF16, kind="ExternalInput")
    # output bias as a [1,128] row: b3[j] at col 32*k+j, used as K=1 lhsT to
    # seed the quad-packed layer-3 PSUM bank (host-built, zero FLOPs)
    b3row_d = nc.dram_tensor("b3row", [1, 128], F32, kind="ExternalInput")
    ones_d = nc.dram_tensor("ones_row", [1, NF], F32, kind="ExternalInput")
    oshape = [2, P_SHARD, CK] if reps == 1 else [reps, 2, P_SHARD, CK]
    out = nc.dram_tensor("out", oshape, F32, kind="ExternalOutput")

    with tile.TileContext(nc) as tc:
        for r in range(reps):
            out_r = out.ap() if reps == 1 else out[r]
            _kernel_body(nc, tc, xgT, pre_wT, pre_b, w1q_d, w2q_d, b1s_d,
                         b2s_d, w3T, b3row_d, ones_d, out_r)
    nc.compile()
    return nc


def _kernel_body(nc, tc, xgT, pre_wT, pre_b, w1q_d, w2q_d, b1s_d, b2s_d,
                 w3T, b3row_d, ones_d, out):
    with (
        tc.tile_pool(name="consts", bufs=2) as consts,
        tc.tile_pool(name="feats", bufs=2) as feats,
        tc.tile_pool(name="work",
                     bufs=int(os.environ.get("WORK_BUFS", "3"))) as work,
        tc.tile_pool(name="psum", bufs=1, space="PSUM") as psum,
    ):
        # ---- weights / biases to SBUF (replicated, loaded once) ----
        def col(ap_1d):  # [n] -> [n,1]
            return ap_1d.rearrange("(n one) -> n one", one=1)

        wpre = [[consts.tile([128, C], MM_DT, name=f"wpre_{i}_{cc}")
                 for cc in range(2)] for i in range(3)]
        bpre = [[consts.tile([128, 1], F32, name=f"bpre_{i}_{oc}")
                 for oc in range(2)] for i in range(3)]
        for i in range(3):
            for cc in range(2):
                nc.sync.dma_start(
                    out=wpre[i][cc][:],
                    in_=pre_wT[i, cc * 128:(cc + 1) * 128, :].bitcast(MM_DT))
            for oc in range(2):
                nc.sync.dma_start(out=bpre[i][oc][:],
                                  in_=col(pre_b[i, oc * 128:(oc + 1) * 128]))
        w1q = [consts.tile([128, 2, 128], FP8, name=f"w1q_{oc}")
               for oc in range(2)]
        w2q = [consts.tile([128, 2, 128], FP8, name=f"w2q_{oc}")
               for oc in range(2)]
        b1s = [consts.tile([128, 1], F32, name=f"b1s_{oc}") for oc in range(2)]
        b2s = [consts.tile([128, 1], F32, name=f"b2s_{oc}") for oc in range(2)]
        for oc in range(2):
            nc.sync.dma_start(out=w1q[oc][:], in_=w1q_d[oc])
            nc.sync.dma_start(out=w2q[oc][:], in_=w2q_d[oc])
            nc.sync.dma_start(out=b1s[oc][:], in_=col(b1s_d[oc]))
            nc.sync.dma_start(out=b2s[oc][:], in_=col(b2s_d[oc]))
        w3 = [consts.tile([128, 2], BF16, name=f"w3_{cc}") for cc in range(2)]
        for cc in range(2):
            nc.sync.dma_start(out=w3[cc][:],
                              in_=w3T[cc * 128:(cc + 1) * 128, :])
        b3row = consts.tile([1, 128], MM_DT, name="b3row")
        nc.sync.dma_start(out=b3row[:], in_=b3row_d.ap().bitcast(MM_DT))
        ones_row = consts.tile([1, NF], MM_DT, name="ones_row")
        nc.sync.dma_start(out=ones_row[:], in_=ones_d.ap().bitcast(MM_DT))

        # ---- stage 1: pre-MLP over the 320 gathered pixel columns ----
        cur = [feats.tile([128, NJ], MM_DT, name=f"xg_{cc}") for cc in range(2)]
        for cc in range(2):
            nc.sync.dma_start(out=cur[cc][:],
                              in_=xgT[cc * 128:(cc + 1) * 128, :].bitcast(MM_DT))
        for i in range(3):
            nxt = [feats.tile([128, NJ], MM_DT if i < 2 else BF16,
                              name=f"feat{i}_{oc}") for oc in range(2)]
            for oc in range(2):
                ps = psum.tile([128, NJ], F32, name=f"ps_s1_{i}_{oc}",
                               tag="ps_a", bufs=3)
                nc.tensor.matmul(
                    out=ps[:],
                    lhsT=wpre[i][0][:, oc * 128:(oc + 1) * 128],
                    rhs=cur[0][:], start=True, stop=False)
                nc.tensor.matmul(
                    out=ps[:],
                    lhsT=wpre[i][1][:, oc * 128:(oc + 1) * 128],
                    rhs=cur[1][:], start=False, stop=True)
                if oc == 0:
                    nc.scalar.activation(
                        out=nxt[oc][:], in_=ps[:],
                        func=AF.Relu if i < 2 else AF.Identity,
                        bias=bpre[i][oc][:, 0:1], scale=1.0)
                elif i < 2:
                    nc.vector.tensor_scalar(
                        out=nxt[oc][:], in0=ps[:],
                        scalar1=bpre[i][oc][:, 0:1], scalar2=0.0,
                        op0=OP.add, op1=OP.max)
                else:
                    nc.vector.tensor_scalar(
                        out=nxt[oc][:], in0=ps[:],
                        scalar1=bpre[i][oc][:, 0:1], scalar2=None,
                        op0=OP.add)
            cur = nxt
        F = cur  # [128, NJ] bf16 x2 chunks; 0:P_SHARD primary, rest compare
        # F is pre-scaled by sqrt(SXX) via host-folded pre_w[2]/pre_b[2]
        # fp32 copy of the primary columns (tensor_scalar scalars must be f32)
        Fp = [feats.tile([128, P_SHARD], F32, name=f"Fp_{cc}")
              for cc in range(2)]
        for cc in range(2):
            nc.vector.tensor_copy(out=Fp[cc][:], in_=F[cc][:, 0:P_SHARD])

        # ---- stage 2: software-pipelined with a 4-deep skew ----
        NG = P_SHARD // PAIR
        if VARIANT == "half":
            NG = NG // 2
        d_t, xx_t, h1_t, h2_t, ps3_t = {}, {}, {}, {}, {}

        # engine maps (tuned on HW via interleaved A/B): 4 squares (s,cc) and
        # 4 drains (l1o0 l1o1 l2o0 l2o1).  GPSIMD cannot touch PSUM, so
        # drains are act/dve only; >1 GPSIMD op per group measurably degrades
        # the pipeline (strict-FIFO Q7 dispatch), so at most one is used.
        # Override via ASSIGN="sq:act,gp,dve,gp;dr:act,dve,act,dve;cp:dve"
        SQ_ENG = ("gpdve",)
        DRAIN_ENG = ("act", "dve", "act", "dve")
        CP_ENG = "act"
        for part in os.environ.get("ASSIGN", "").split(";"):
            if part.startswith("sq:"):
                SQ_ENG = tuple(part[3:].split(","))
            elif part.startswith("dr:"):
                DRAIN_ENG = tuple(part[3:].split(","))
            elif part.startswith("cp:"):
                CP_ENG = part[3:]
        L3_DMA = False  # bass dma_start cannot read PSUM
        XX_MODE = os.environ.get("XX_MODE", "mix")

        def stage_xx(g):
            if XX_MODE == "act1":
                # one fused ACT op per slice: (xp - xc)^2 -> fp8
                xx = work.tile([128, 2, NF], FP8, name="xx", tag="xx")
                for s in range(PAIR):
                    p = g * PAIR + s
                    for cc in range(2):
                        nc.scalar.activation(
                            out=xx[:, cc, s * CK:(s + 1) * CK],
                            in_=F[cc][:, P_SHARD:NJ],
                            func=AF.Square,
                            bias=Fp[cc][:, p:p + 1], scale=-1.0)
                xx_t[g] = xx
                return
            if XX_MODE == "mix":
                # chunk 0: fused ACT Square per primary; chunk 1: DVE 4x subs
                # into a bf16 d-strip, then one merged FD-512 square.
                xx = work.tile([128, 2, NF], FP8, name="xx", tag="xx")
                d = work.tile([128, NF], BF16, name="d", tag="d")
                for s in range(PAIR):
                    p = g * PAIR + s
                    nc.scalar.activation(
                        out=xx[:, 0, s * CK:(s + 1) * CK],
                        in_=F[0][:, P_SHARD:NJ],
                        func=AF.Square,
                        bias=Fp[0][:, p:p + 1], scale=-1.0)
                    nc.vector.tensor_scalar(
                        out=d[:, s * CK:(s + 1) * CK],
                        in0=F[1][:, P_SHARD:NJ],
                        scalar1=Fp[1][:, p:p + 1], scalar2=None,
                        op0=OP.subtract)
                if SQ_ENG[0] == "act":
                    nc.scalar.activation(out=xx[:, 1, :], in_=d[:],
                                         func=AF.Square)
                elif SQ_ENG[0] == "gpdve":
                    nc.gpsimd.tensor_tensor(out=xx[:, 1, 0:CK],
                                            in0=d[:, 0:CK], in1=d[:, 0:CK],
                                            op=OP.mult)
                    nc.vector.tensor_tensor(out=xx[:, 1, CK:NF],
                                            in0=d[:, CK:NF], in1=d[:, CK:NF],
                                            op=OP.mult)
                elif SQ_ENG[0] == "gp":
                    # two FD-256 GP ops (cheaper than one FD-512 on the Q7s)
                    for s in range(PAIR):
                        sl = slice(s * CK, (s + 1) * CK)
                        nc.gpsimd.tensor_tensor(out=xx[:, 1, sl],
                                                in0=d[:, sl], in1=d[:, sl],
                                                op=OP.mult)
                else:
                    nc.vector.tensor_tensor(out=xx[:, 1, :], in0=d[:],
                                            in1=d[:], op=OP.mult)
                xx_t[g] = xx
                return
            # d[:, cc, s*CK:(s+1)*CK] = F[cc][:, cmp] - F[cc][:, p_s]  (bf16,
            # DVE tensor_scalar hits the 4x path), then squared into fp8.
            d = work.tile([128, 2, NF], BF16, name="d", tag="d")
            for s in range(PAIR):
                p = g * PAIR + s
                for cc in range(2):
                    nc.vector.tensor_scalar(
                        out=d[:, cc, s * CK:(s + 1) * CK],
                        in0=F[cc][:, P_SHARD:NJ],
                        scalar1=Fp[cc][:, p:p + 1], scalar2=None,
                        op0=OP.subtract)
            d_t[g] = d

        def stage_sq(g):
            if XX_MODE in ("act1", "mix"):
                return
            d = d_t.pop(g)
            xx = work.tile([128, 2, NF], FP8, name="xx", tag="xx")
            for s in range(PAIR):
                for cc in range(2):
                    src = d[:, cc, s * CK:(s + 1) * CK]
                    dst = xx[:, cc, s * CK:(s + 1) * CK]
                    eng = SQ_ENG[2 * s + cc]
                    if eng == "act":
                        nc.scalar.activation(out=dst, in_=src, func=AF.Square)
                    elif eng == "dve":
                        nc.vector.tensor_tensor(out=dst, in0=src, in1=src,
                                                op=OP.mult)
                    else:
                        nc.gpsimd.tensor_tensor(out=dst, in0=src, in1=src,
                                                op=OP.mult)
            xx_t[g] = xx

        def drain(which, out_ap, ps, bias):
            # relu(psum + bias) -> out (fp8/bf16); scales are pre-folded
            if which == "act":
                nc.scalar.activation(out=out_ap, in_=ps[:], func=AF.Relu,
                                     bias=bias[:, 0:1], scale=1.0)
            else:
                nc.vector.tensor_scalar(
                    out=out_ap, in0=ps[:], scalar1=bias[:, 0:1], scalar2=0.0,
                    op0=OP.add, op1=OP.max)

        def stage_l1(g):
            xx = xx_t.pop(g)
            h1 = work.tile([128, 2, NF], FP8, name="h1", tag="h1")
            for oc in range(2):
                ps = psum.tile([128, NF], F32, name=f"ps_l1_{oc}",
                               tag="ps_a", bufs=3)
                nc.tensor.matmul(out=ps[:], lhsT=w1q[oc][:], rhs=xx[:],
                                 perf_mode=DR, start=True, stop=True)
                drain(DRAIN_ENG[oc], h1[:, oc, :], ps, b1s[oc])
            h1_t[g] = h1

        def stage_l2(g):
            h1 = h1_t.pop(g)
            h2 = work.tile([128, 2, NF], BF16, name="h2", tag="h2")
            for oc in range(2):
                ps = psum.tile([128, NF], F32, name=f"ps_l2_{oc}",
                               tag="ps_b", bufs=3)
                nc.tensor.matmul(out=ps[:], lhsT=w2q[oc][:], rhs=h1[:],
                                 perf_mode=DR, start=True, stop=True)
                drain(DRAIN_ENG[2 + oc], h2[:, oc, :], ps, b2s[oc])
            h2_t[g] = h2

        def stage_l3(g):
            # layer 3 (c->2): quad-packed into one PSUM bank -- group g%QUAD
            # computes at array col-group k, writing PSUM partitions
            # 32k..32k+1. Bias pre-seeded by a K=1 matmul that defines every
            # row. The quad drains by DMA straight from PSUM (or one DVE
            # copy + 4 DMAs under VARIANT=l3copy).
            hcur = h2_t.pop(g)
            k = g % QUAD
            if k == 0:
                ps3 = psum.tile([128, NF], F32, name="ps3", tag="ps3", bufs=2)
                nc.tensor.matmul(out=ps3[:], lhsT=b3row[:], rhs=ones_row[:],
                                 start=True, stop=True)
                ps3_t[g // QUAD] = ps3
            ps3 = ps3_t[g // QUAD]
            nc.tensor.matmul(out=ps3[32 * k:32 * k + 2, :], lhsT=w3[0][:],
                             rhs=hcur[:, 0, :], tile_position=(0, 32 * k),
                             start=False, stop=True, skip_group_check=True)
            nc.tensor.matmul(out=ps3[32 * k:32 * k + 2, :], lhsT=w3[1][:],
                             rhs=hcur[:, 1, :], tile_position=(0, 32 * k),
                             start=False, stop=True, skip_group_check=True)
            if k == QUAD - 1:
                ps3_t.pop(g // QUAD)
                qb = (g // QUAD) * QUAD * PAIR
                if L3_DMA:
                    for kk in range(QUAD):
                        nc.sync.dma_start(
                            out=out[:, qb + kk * PAIR:qb + (kk + 1) * PAIR, :],
                            in_=ps3[32 * kk:32 * kk + 2, :].rearrange(
                                "j (s q) -> j s q", s=PAIR))
                else:
                    ob = work.tile([128, NF], F32, name="ob", tag="ob", bufs=2)
                    if CP_ENG == "act":
                        nc.scalar.activation(out=ob[:], in_=ps3[:],
                                             func=AF.Copy)
                    else:
                        nc.vector.tensor_copy(out=ob[:], in_=ps3[:])
                    for kk in range(QUAD):
                        nc.sync.dma_start(
                            out=out[:, qb + kk * PAIR:qb + (kk + 1) * PAIR, :],
                            in_=ob[32 * kk:32 * kk + 2, :].rearrange(
                                "j (s q) -> j s q", s=PAIR))

        # Emit the independent producer stages (xx subs, squares) BEFORE the
        # PSUM drains each iteration: ACT/DVE are strict-FIFO, so a drain
        # waiting on PE at queue head would head-of-line-block the subs.
        for i in range(NG + 4):
            if i < NG:
                stage_xx(i)
            if 1 <= i < NG + 1:
                stage_sq(i - 1)
            if 2 <= i < NG + 2:
                stage_l1(i - 2)
            if 3 <= i < NG + 3:
                stage_l2(i - 3)
            if i >= 4:
                stage_l3(i - 4)


_NC_CACHE = {}


def _get_nc():
    if "nc" not in _NC_CACHE:
        _NC_CACHE["nc"] = _build_nc()
    return _NC_CACHE["nc"]


def _shard_inputs(x, primary_indices, compare_indices, pre_w, pre_b,
                  post_w, post_b, post_out_w, post_out_b):
    """Host-side sharding: per-core index slicing + row gather of x (pure
    data movement -- the pre-MLP commutes with the gather), weight
    transposes, and fp8 scale folding. Returns the 8 per-core input maps."""
    import ml_dtypes
    E4M3 = ml_dtypes.float8_e4m3
    x = np.asarray(x, dtype=np.float32)
    x_rows = np.ascontiguousarray(x.transpose(0, 2, 3, 1)).reshape(B * H * W, C)
    pre_wT = np.asarray(pre_w, dtype=np.float32).transpose(0, 2, 1).copy()
    pre_b = np.asarray(pre_b, dtype=np.float32).copy()
    # fold sqrt(SXX) into the last pre layer so F is pre-scaled for xx fp8
    rt = np.float32(np.sqrt(SXX))
    pre_wT[2] *= rt
    pre_b[2] *= rt

    post_w = np.asarray(post_w, dtype=np.float32)
    post_b = np.asarray(post_b, dtype=np.float32)
    # DoubleRow-interleaved fp8 weights [oc_chunk, k, k_chunk, m]:
    # arr[o,c] -> o=(j,m), c=(i,k) -> [j,k,i,m]
    def dr_pack(wmat, scale):
        a = (wmat * scale).reshape(2, 128, 2, 128)   # [j, m, i, k]
        return np.ascontiguousarray(a.transpose(0, 3, 2, 1)).astype(E4M3)

    w1q = dr_pack(post_w[0], S1 / SXX)
    w2q = dr_pack(post_w[1], A2)
    b1s = np.ascontiguousarray((post_b[0] * S1).reshape(2, 128))
    b2s = np.ascontiguousarray((post_b[1] * (S1 * A2)).reshape(2, 128))

    w3T = np.ascontiguousarray(
        (np.asarray(post_out_w, dtype=np.float32) / (S1 * A2)).T
    ).astype(ml_dtypes.bfloat16)
    b3 = np.asarray(post_out_b, dtype=np.float32)
    b3row = np.zeros((1, 128), dtype=np.float32)
    for k in range(QUAD):
        b3row[0, 32 * k:32 * k + 2] = b3
    primary_indices = np.asarray(primary_indices)
    compare_indices = np.asarray(compare_indices)

    in_maps = []
    for core in range(N_CORES):
        b = core // CORES_PER_BATCH
        ps = (core % CORES_PER_BATCH) * P_SHARD
        rows = np.concatenate([
            primary_indices[b, ps:ps + P_SHARD].astype(np.int64),
            compare_indices[b].astype(np.int64),
        ])
        xg_T = np.ascontiguousarray(x_rows[rows].T)  # [C, NJ]
        in_maps.append({
            "xgT": xg_T,
            "pre_wT": pre_wT,
            "pre_b": pre_b,
            "w1q": w1q,
            "w2q": w2q,
            "b1s": b1s,
            "b2s": b2s,
            "w3T": w3T,
            "b3row": b3row,
            "ones_row": np.ones((1, NF), dtype=np.float32),
        })
    return in_maps


def _unshard_output(results):
    out = np.empty((B, 2, PK, CK), dtype=np.float32)
    for core in range(N_CORES):
        b = core // CORES_PER_BATCH
        ps = (core % CORES_PER_BATCH) * P_SHARD
        out[b, :, ps:ps + P_SHARD, :] = results[core]["out"]
    return out


def kernel(x, primary_indices, compare_indices, pre_w, pre_b,
           post_w, post_b, post_out_w, post_out_b):
    in_maps = _shard_inputs(x, primary_indices, compare_indices, pre_w, pre_b,
                            post_w, post_b, post_out_w, post_out_b)
    nc = _get_nc()
    res = run_bass_kernel_spmd(nc, in_maps, core_ids=list(range(N_CORES)))
    return _unshard_output(res.results)



# revision 4
# speedup vs baseline: 2.9791x; 2.9791x over previous
"""Trainium2 Bass kernel for nn_DenseEdgeModel (gnn_message_passing).

Reference computation (all 1x1 convs == per-pixel matmuls over channels):
    h    = MLP3(x)                    # 3x (c->c) with ReLU between    [B,C,H,W]
    flat = h as [B*H*W, C]
    xp   = flat[primary_idx]          # [B,PK,C] -> [B,C,PK]
    xc   = flat[compare_idx]          # [B,CK,C] -> [B,C,CK]
    xx   = (xp[..,:,None]-xc[..,None,:])**2          # [B,C,PK,CK]
    g    = relu(W1@xx+b1); g = relu(W2@g+b2)         # over C
    out  = W3@g + b3                  # [B,2,PK,CK]

Sharding (8 cores): data-parallel over batch (4 cores per batch), and the
PK axis split 4-ways within each batch -> each core owns 64 primary
indices of one batch and all 256 compare indices of that batch.

Because the pre-MLP is per-pixel, gather commutes with it:
MLP(x)[idx] == MLP(x[idx]). The host therefore only *slices* (gathers
rows of x for each core's indices and transposes to channel-major).

Device kernel per core:
  stage 1: pre-MLP on the 320 gathered pixel columns [c,320] in fp32r
           (drains split ACT/DVE).  The last layer's weights/bias are
           host-folded by sqrt(SXX) so its bf16 output F is pre-scaled
           for fp8 xx quantization.
  stage 2: per pair of primary columns p ("group"):
    xx   [128,2,512] fp8: channel-chunk 0 via fused ACT Square
           (bias=-xp per primary, FD=256 x2); chunk 1 via DVE
           tensor_scalar subtract (bf16, hits the 4x DVE mode) into a
           bf16 strip, then squared fp8 by one GPSIMD + one DVE
           tensor_tensor (exactly one GPSIMD op per group -- more
           degrades the pipeline).
    l1/l2: ONE fp8e4 DoubleRow matmul per 128-out-chunk (K=256 in a
           single pass at ~2x rate, HW-measured ~230ns for K=256xN=512);
           PSUM holds scale-folded pre-activations; drains
           relu(psum + scaled bias) split ACT/DVE write fp8 (l1) / bf16
           (l2) tiles directly (scales engineered host-side so no
           per-drain rescale op is needed).
    l3   (c->2): bf16 quad-packed into one PSUM bank via tile_position
           col-tiling (bias pre-seeded by a K=1 matmul); one ACT copy +
           4 DMAs drain the quad.  w3 is host-folded by 1/(S1*A2).

Quantization scheme (validated vs reference: rel_err ~1.2e-2 < 2e-2):
  F' = sqrt(SXX)*F;  xx_q = e4m3(SXX*xx);  W1q = e4m3((S1/SXX)*W1)
  psum1 = S1*Z1;  h1_q = e4m3(relu(psum1 + S1*b1)) = e4m3(S1*h1)
  W2q = e4m3(A2*W2);  psum2 = S1*A2*Z2;  h2 = bf16(relu(psum2+S1*A2*b2))
  out = (w3/(S1*A2))@h2 + b3
(e4m3 here is TRN FP8_EXP4: max normal +-240, so scales keep peaks
well under 240.)
"""

import os

import numpy as np

import concourse.bass as bass
import concourse.tile as tile
from concourse import bacc, mybir
from concourse.bass_utils import run_bass_kernel_spmd

# Problem constants (hardcoded per harness contract)
B, C, H, W = 2, 256, 32, 32
PK, CK = 256, 256
N_CORES = 8
CORES_PER_BATCH = N_CORES // B          # 4
P_SHARD = PK // CORES_PER_BATCH         # 64 primary indices per core
NJ = P_SHARD + CK                       # 320 gathered pixel columns per core
PAIR = 2                                # primary columns per stage-2 group
NF = PAIR * CK                          # 512 = stage-2 matmul free dim
QUAD = 4                                # groups sharing one l3 PSUM bank
F32 = mybir.dt.float32
BF16 = mybir.dt.bfloat16
FP8 = mybir.dt.float8e4
MM_DT = mybir.dt.float32r               # stage-1 full-rate fp32 streaming
DR = mybir.MatmulPerfMode.DoubleRow
VARIANT = os.environ.get("KERNEL_VARIANT", "")
AF = mybir.ActivationFunctionType
OP = mybir.AluOpType

# fp8 scale scheme (host-folded; see module docstring)
SXX = 128.0
S1 = 1024.0
A2 = 2048.0


def _build_nc(reps=1):
    nc = bacc.Bacc("TRN2", target_bir_lowering=False, debug=False)

    xgT = nc.dram_tensor("xgT", [C, NJ], F32, kind="ExternalInput")
    pre_wT = nc.dram_tensor("pre_wT", [3, C, C], F32, kind="ExternalInput")
    pre_b = nc.dram_tensor("pre_b", [3, C], F32, kind="ExternalInput")
    # DoubleRow-interleaved fp8 post weights: [oc_chunk, k, k_chunk, m]
    w1q_d = nc.dram_tensor("w1q", [2, 128, 2, 128], FP8, kind="ExternalInput")
    w2q_d = nc.dram_tensor("w2q", [2, 128, 2, 128], FP8, kind="ExternalInput")
    b1s_d = nc.dram_tensor("b1s", [2, 128], F32, kind="ExternalInput")
    b2s_d = nc.dram_tensor("b2s", [2, 128], F32, kind="ExternalInput")
    w3T = nc.dram_tensor("w3T", [C, 2], BF16, kind="ExternalInput")
    # output bias as a [1,128] row: b3[j] at col 32*k+j, used as K=1 lhsT to
    # seed the quad-packed layer-3 PSUM bank (host-built, zero FLOPs)
    b3row_d = nc.dram_tensor("b3row", [1, 128], F32, kind="ExternalInput")
    ones_d = nc.dram_tensor("ones_row", [1, NF], F32, kind="ExternalInput")
    oshape = [2, P_SHARD, CK] if reps == 1 else [reps, 2, P_SHARD, CK]
    out = nc.dram_tensor("out", oshape, F32, kind="ExternalOutput")

    with tile.TileContext(nc) as tc:
        for r in range(reps):
            out_r = out.ap() if reps == 1 else out[r]
            _kernel_body(nc, tc, xgT, pre_wT, pre_b, w1q_d, w2q_d, b1s_d,
                         b2s_d, w3T, b3row_d, ones_d, out_r)
    nc.compile()
    return nc


def _kernel_body(nc, tc, xgT, pre_wT, pre_b, w1q_d, w2q_d, b1s_d, b2s_d,
                 w3T, b3row_d, ones_d, out):
    with (
        tc.tile_pool(name="consts", bufs=2) as consts,
        tc.tile_pool(name="feats", bufs=2) as feats,
        tc.tile_pool(name="work",
                     bufs=int(os.environ.get("WORK_BUFS", "3"))) as work,
        tc.tile_pool(name="psum", bufs=1, space="PSUM") as psum,
    ):
        # ---- weights / biases to SBUF (replicated, loaded once) ----
        def col(ap_1d):  # [n] -> [n,1]
            return ap_1d.rearrange("(n one) -> n one", one=1)

        wpre = [[consts.tile([128, C], MM_DT, name=f"wpre_{i}_{cc}")
                 for cc in range(2)] for i in range(3)]
        bpre = [[consts.tile([128, 1], F32, name=f"bpre_{i}_{oc}")
                 for oc in range(2)] for i in range(3)]
        for i in range(3):
            for cc in range(2):
                nc.sync.dma_start(
                    out=wpre[i][cc][:],
                    in_=pre_wT[i, cc * 128:(cc + 1) * 128, :].bitcast(MM_DT))
            for oc in range(2):
                nc.sync.dma_start(out=bpre[i][oc][:],
                                  in_=col(pre_b[i, oc * 128:(oc + 1) * 128]))
        w1q = [consts.tile([128, 2, 128], FP8, name=f"w1q_{oc}")
               for oc in range(2)]
        w2q = [consts.tile([128, 2, 128], FP8, name=f"w2q_{oc}")
               for oc in range(2)]
        b1s = [consts.tile([128, 1], F32, name=f"b1s_{oc}") for oc in range(2)]
        b2s = [consts.tile([128, 1], F32, name=f"b2s_{oc}") for oc in range(2)]
        for oc in range(2):
            nc.sync.dma_start(out=w1q[oc][:], in_=w1q_d[oc])
            nc.sync.dma_start(out=w2q[oc][:], in_=w2q_d[oc])
            nc.sync.dma_start(out=b1s[oc][:], in_=col(b1s_d[oc]))
            nc.sync.dma_start(out=b2s[oc][:], in_=col(b2s_d[oc]))
        w3 = [consts.tile([128, 2], BF16, name=f"w3_{cc}") for cc in range(2)]
        for cc in range(2):
            nc.sync.dma_start(out=w3[cc][:],
                              in_=w3T[cc * 128:(cc + 1) * 128, :])
        b3row = consts.tile([1, 128], MM_DT, name="b3row")
        nc.sync.dma_start(out=b3row[:], in_=b3row_d.ap().bitcast(MM_DT))
        ones_row = consts.tile([1, NF], MM_DT, name="ones_row")
        nc.sync.dma_start(out=ones_row[:], in_=ones_d.ap().bitcast(MM_DT))

        # ---- stage 1: pre-MLP over the 320 gathered pixel columns ----
        cur = [feats.tile([128, NJ], MM_DT, name=f"xg_{cc}") for cc in range(2)]
        for cc in range(2):
            nc.sync.dma_start(out=cur[cc][:],
                              in_=xgT[cc * 128:(cc + 1) * 128, :].bitcast(MM_DT))
        for i in range(3):
            nxt = [feats.tile([128, NJ], MM_DT if i < 2 else BF16,
                              name=f"feat{i}_{oc}") for oc in range(2)]
            for oc in range(2):
                ps = psum.tile([128, NJ], F32, name=f"ps_s1_{i}_{oc}",
                               tag="ps_a", bufs=3)
                nc.tensor.matmul(
                    out=ps[:],
                    lhsT=wpre[i][0][:, oc * 128:(oc + 1) * 128],
                    rhs=cur[0][:], start=True, stop=False)
                nc.tensor.matmul(
                    out=ps[:],
                    lhsT=wpre[i][1][:, oc * 128:(oc + 1) * 128],
                    rhs=cur[1][:], start=False, stop=True)
                if oc == 0:
                    nc.scalar.activation(
                        out=nxt[oc][:], in_=ps[:],
                        func=AF.Relu if i < 2 else AF.Identity,
                        bias=bpre[i][oc][:, 0:1], scale=1.0)
                elif i < 2:
                    nc.vector.tensor_scalar(
                        out=nxt[oc][:], in0=ps[:],
                        scalar1=bpre[i][oc][:, 0:1], scalar2=0.0,
                        op0=OP.add, op1=OP.max)
                else:
                    nc.vector.tensor_scalar(
                        out=nxt[oc][:], in0=ps[:],
                        scalar1=bpre[i][oc][:, 0:1], scalar2=None,
                        op0=OP.add)
            cur = nxt
        F = cur  # [128, NJ] bf16 x2 chunks; 0:P_SHARD primary, rest compare
        # F is pre-scaled by sqrt(SXX) via host-folded pre_w[2]/pre_b[2]
        # fp32 copy of the primary columns (tensor_scalar scalars must be f32)
        Fp = [feats.tile([128, P_SHARD], F32, name=f"Fp_{cc}")
              for cc in range(2)]
        for cc in range(2):
            nc.vector.tensor_copy(out=Fp[cc][:], in_=F[cc][:, 0:P_SHARD])

        # ---- stage 2: software-pipelined with a 4-deep skew ----
        NG = P_SHARD // PAIR
        if VARIANT == "half":
            NG = NG // 2
        d_t, xx_t, h1_t, h2_t, ps3_t = {}, {}, {}, {}, {}

        # engine maps (tuned on HW via interleaved A/B): 4 squares (s,cc) and
        # 4 drains (l1o0 l1o1 l2o0 l2o1).  GPSIMD cannot touch PSUM, so
        # drains are act/dve only; >1 GPSIMD op per group measurably degrades
        # the pipeline (strict-FIFO Q7 dispatch), so at most one is used.
        # Override via ASSIGN="sq:act,gp,dve,gp;dr:act,dve,act,dve;cp:dve"
        SQ_ENG = ("gpdve",)
        DRAIN_ENG = ("act", "dve", "act", "dve")
        CP_ENG = "act"
        for part in os.environ.get("ASSIGN", "").split(";"):
            if part.startswith("sq:"):
                SQ_ENG = tuple(part[3:].split(","))
            elif part.startswith("dr:"):
                DRAIN_ENG = tuple(part[3:].split(","))
            elif part.startswith("cp:"):
                CP_ENG = part[3:]
        L3_DMA = False  # bass dma_start cannot read PSUM
        XX_MODE = os.environ.get("XX_MODE", "mix")

        def stage_xx(g):
            if XX_MODE == "act1":
                # one fused ACT op per slice: (xp - xc)^2 -> fp8
                xx = work.tile([128, 2, NF], FP8, name="xx", tag="xx")
                for s in range(PAIR):
                    p = g * PAIR + s
                    for cc in range(2):
                        nc.scalar.activation(
                            out=xx[:, cc, s * CK:(s + 1) * CK],
                            in_=F[cc][:, P_SHARD:NJ],
                            func=AF.Square,
                            bias=Fp[cc][:, p:p + 1], scale=-1.0)
                xx_t[g] = xx
                return
            if XX_MODE == "mix":
                # chunk 0: fused ACT Square per primary; chunk 1: DVE 4x subs
                # into a bf16 d-strip, then one merged FD-512 square.
                xx = work.tile([128, 2, NF], FP8, name="xx", tag="xx")
                d = work.tile([128, NF], BF16, name="d", tag="d")
                for s in range(PAIR):
                    p = g * PAIR + s
                    nc.scalar.activation(
                        out=xx[:, 0, s * CK:(s + 1) * CK],
                        in_=F[0][:, P_SHARD:NJ],
                        func=AF.Square,
                        bias=Fp[0][:, p:p + 1], scale=-1.0)
                    nc.vector.tensor_scalar(
                        out=d[:, s * CK:(s + 1) * CK],
                        in0=F[1][:, P_SHARD:NJ],
                        scalar1=Fp[1][:, p:p + 1], scalar2=None,
                        op0=OP.subtract)
                if SQ_ENG[0] == "act":
                    nc.scalar.activation(out=xx[:, 1, :], in_=d[:],
                                         func=AF.Square)
                elif SQ_ENG[0] == "gpdve":
                    nc.gpsimd.tensor_tensor(out=xx[:, 1, 0:CK],
                                            in0=d[:, 0:CK], in1=d[:, 0:CK],
                                            op=OP.mult)
                    nc.vector.tensor_tensor(out=xx[:, 1, CK:NF],
                                            in0=d[:, CK:NF], in1=d[:, CK:NF],
                                            op=OP.mult)
                elif SQ_ENG[0] == "gp":
                    # two FD-256 GP ops (cheaper than one FD-512 on the Q7s)
                    for s in range(PAIR):
                        sl = slice(s * CK, (s + 1) * CK)
                        nc.gpsimd.tensor_tensor(out=xx[:, 1, sl],
                                                in0=d[:, sl], in1=d[:, sl],
                                                op=OP.mult)
                else:
                    nc.vector.tensor_tensor(out=xx[:, 1, :], in0=d[:],
                                            in1=d[:], op=OP.mult)
                xx_t[g] = xx
                return
            # d[:, cc, s*CK:(s+1)*CK] = F[cc][:, cmp] - F[cc][:, p_s]  (bf16,
            # DVE tensor_scalar hits the 4x path), then squared into fp8.
            d = work.tile([128, 2, NF], BF16, name="d", tag="d")
            for s in range(PAIR):
                p = g * PAIR + s
                for cc in range(2):
                    nc.vector.tensor_scalar(
                        out=d[:, cc, s * CK:(s + 1) * CK],
                        in0=F[cc][:, P_SHARD:NJ],
                        scalar1=Fp[cc][:, p:p + 1], scalar2=None,
                        op0=OP.subtract)
            d_t[g] = d

        def stage_sq(g):
            if XX_MODE in ("act1", "mix"):
                return
            d = d_t.pop(g)
            xx = work.tile([128, 2, NF], FP8, name="xx", tag="xx")
            for s in range(PAIR):
                for cc in range(2):
                    src = d[:, cc, s * CK:(s + 1) * CK]
                    dst = xx[:, cc, s * CK:(s + 1) * CK]
                    eng = SQ_ENG[2 * s + cc]
                    if eng == "act":
                        nc.scalar.activation(out=dst, in_=src, func=AF.Square)
                    elif eng == "dve":
                        nc.vector.tensor_tensor(out=dst, in0=src, in1=src,
                                                op=OP.mult)
                    else:
                        nc.gpsimd.tensor_tensor(out=dst, in0=src, in1=src,
                                                op=OP.mult)
            xx_t[g] = xx

        def drain(which, out_ap, ps, bias):
            # relu(psum + bias) -> out (fp8/bf16); scales are pre-folded
            if which == "act":
                nc.scalar.activation(out=out_ap, in_=ps[:], func=AF.Relu,
                                     bias=bias[:, 0:1], scale=1.0)
            else:
                nc.vector.tensor_scalar(
                    out=out_ap, in0=ps[:], scalar1=bias[:, 0:1], scalar2=0.0,
                    op0=OP.add, op1=OP.max)

        def stage_l1(g):
            xx = xx_t.pop(g)
            h1 = work.tile([128, 2, NF], FP8, name="h1", tag="h1")
            for oc in range(2):
                ps = psum.tile([128, NF], F32, name=f"ps_l1_{oc}",
                               tag="ps_a", bufs=3)
                nc.tensor.matmul(out=ps[:], lhsT=w1q[oc][:], rhs=xx[:],
                                 perf_mode=DR, start=True, stop=True)
                drain(DRAIN_ENG[oc], h1[:, oc, :], ps, b1s[oc])
            h1_t[g] = h1

        def stage_l2(g):
            h1 = h1_t.pop(g)
            # bufs=6: up to 4 quad members live awaiting the l3 burst, plus
            # pipeline slack so the l2 drain never waits on the burst's reads
            h2 = work.tile([128, 2, NF], BF16, name="h2", tag="h2", bufs=6)
            for oc in range(2):
                ps = psum.tile([128, NF], F32, name=f"ps_l2_{oc}",
                               tag="ps_b", bufs=3)
                nc.tensor.matmul(out=ps[:], lhsT=w2q[oc][:], rhs=h1[:],
                                 perf_mode=DR, start=True, stop=True)
                drain(DRAIN_ENG[2 + oc], h2[:, oc, :], ps, b2s[oc])
            h2_t[g] = h2

        def stage_l3(q):
            # layer 3 (c->2) for a whole quad of 4 groups, emitted as ONE
            # back-to-back PE burst: bias seed (K=1, defines every row), then
            # the 8 col-tiled matmuls ordered cc-major so the 4 distinct
            # col-groups (tile_position=(0,32k)) stream CONCURRENTLY on
            # disjoint 32-col strips of the array (per-subarray overlap,
            # ~2 serialized rounds of 512 cols instead of 8).  Math and
            # start/stop flags are identical to the per-group form.
            ps3 = psum.tile([128, NF], F32, name="ps3", tag="ps3", bufs=2)
            nc.tensor.matmul(out=ps3[:], lhsT=b3row[:], rhs=ones_row[:],
                             start=True, stop=True)
            hq = [h2_t.pop(q * QUAD + k) for k in range(QUAD)]
            for cc in range(2):
                for k in range(QUAD):
                    nc.tensor.matmul(
                        out=ps3[32 * k:32 * k + 2, :], lhsT=w3[cc][:],
                        rhs=hq[k][:, cc, :], tile_position=(0, 32 * k),
                        start=False, stop=True, skip_group_check=True)
            if True:
                qb = q * QUAD * PAIR
                if L3_DMA:
                    for kk in range(QUAD):
                        nc.sync.dma_start(
                            out=out[:, qb + kk * PAIR:qb + (kk + 1) * PAIR, :],
                            in_=ps3[32 * kk:32 * kk + 2, :].rearrange(
                                "j (s q) -> j s q", s=PAIR))
                else:
                    ob = work.tile([128, NF], F32, name="ob", tag="ob", bufs=2)
                    if CP_ENG == "act":
                        nc.scalar.activation(out=ob[:], in_=ps3[:],
                                             func=AF.Copy)
                    else:
                        nc.vector.tensor_copy(out=ob[:], in_=ps3[:])
                    for kk in range(QUAD):
                        nc.sync.dma_start(
                            out=out[:, qb + kk * PAIR:qb + (kk + 1) * PAIR, :],
                            in_=ob[32 * kk:32 * kk + 2, :].rearrange(
                                "j (s q) -> j s q", s=PAIR))

        # Emit the independent producer stages (xx subs, squares) BEFORE the
        # PSUM drains each iteration: ACT/DVE are strict-FIFO, so a drain
        # waiting on PE at queue head would head-of-line-block the subs.
        for i in range(NG + 4):
            if i < NG:
                stage_xx(i)
            if 1 <= i < NG + 1:
                stage_sq(i - 1)
            if 2 <= i < NG + 2:
                stage_l1(i - 2)
            if 3 <= i < NG + 3:
                stage_l2(i - 3)
            # l3 fires once per quad, after the whole quad's h2 is emitted:
            # quad q's last member (4q+3) finishes stage_l2 at i = 4q+6
            if i >= 7 and (i - 7) % QUAD == 0:
                stage_l3((i - 7) // QUAD)


_NC_CACHE = {}


def _get_nc():
    if "nc" not in _NC_CACHE:
        _NC_CACHE["nc"] = _build_nc()
    return _NC_CACHE["nc"]


def _shard_inputs(x, primary_indices, compare_indices, pre_w, pre_b,
                  post_w, post_b, post_out_w, post_out_b):
    """Host-side sharding: per-core index slicing + row gather of x (pure
    data movement -- the pre-MLP commutes with the gather), weight
    transposes, and fp8 scale folding. Returns the 8 per-core input maps."""
    import ml_dtypes
    E4M3 = ml_dtypes.float8_e4m3
    x = np.asarray(x, dtype=np.float32)
    x_rows = np.ascontiguousarray(x.transpose(0, 2, 3, 1)).reshape(B * H * W, C)
    pre_wT = np.asarray(pre_w, dtype=np.float32).transpose(0, 2, 1).copy()
    pre_b = np.asarray(pre_b, dtype=np.float32).copy()
    # fold sqrt(SXX) into the last pre layer so F is pre-scaled for xx fp8
    rt = np.float32(np.sqrt(SXX))
    pre_wT[2] *= rt
    pre_b[2] *= rt

    post_w = np.asarray(post_w, dtype=np.float32)
    post_b = np.asarray(post_b, dtype=np.float32)
    # DoubleRow-interleaved fp8 weights [oc_chunk, k, k_chunk, m]:
    # arr[o,c] -> o=(j,m), c=(i,k) -> [j,k,i,m]
    def dr_pack(wmat, scale):
        a = (wmat * scale).reshape(2, 128, 2, 128)   # [j, m, i, k]
        return np.ascontiguousarray(a.transpose(0, 3, 2, 1)).astype(E4M3)

    w1q = dr_pack(post_w[0], S1 / SXX)
    w2q = dr_pack(post_w[1], A2)
    b1s = np.ascontiguousarray((post_b[0] * S1).reshape(2, 128))
    b2s = np.ascontiguousarray((post_b[1] * (S1 * A2)).reshape(2, 128))

    w3T = np.ascontiguousarray(
        (np.asarray(post_out_w, dtype=np.float32) / (S1 * A2)).T
    ).astype(ml_dtypes.bfloat16)
    b3 = np.asarray(post_out_b, dtype=np.float32)
    b3row = np.zeros((1, 128), dtype=np.float32)
    for k in range(QUAD):
        b3row[0, 32 * k:32 * k + 2] = b3
    primary_indices = np.asarray(primary_indices)
    compare_indices = np.asarray(compare_indices)

    in_maps = []
    for core in range(N_CORES):
        b = core // CORES_PER_BATCH
        ps = (core % CORES_PER_BATCH) * P_SHARD
        rows = np.concatenate([
            primary_indices[b, ps:ps + P_SHARD].astype(np.int64),
            compare_indices[b].astype(np.int64),
        ])
        xg_T = np.ascontiguousarray(x_rows[rows].T)  # [C, NJ]
        in_maps.append({
            "xgT": xg_T,
            "pre_wT": pre_wT,
            "pre_b": pre_b,
            "w1q": w1q,
            "w2q": w2q,
            "b1s": b1s,
            "b2s": b2s,
            "w3T": w3T,
            "b3row": b3row,
            "ones_row": np.ones((1, NF), dtype=np.float32),
        })
    return in_maps


def _unshard_output(results):
    out = np.empty((B, 2, PK, CK), dtype=np.float32)
    for core in range(N_CORES):
        b = core // CORES_PER_BATCH
        ps = (core % CORES_PER_BATCH) * P_SHARD
        out[b, :, ps:ps + P_SHARD, :] = results[core]["out"]
    return out


def kernel(x, primary_indices, compare_indices, pre_w, pre_b,
           post_w, post_b, post_out_w, post_out_b):
    in_maps = _shard_inputs(x, primary_indices, compare_indices, pre_w, pre_b,
                            post_w, post_b, post_out_w, post_out_b)
    nc = _get_nc()
    res = run_bass_kernel_spmd(nc, in_maps, core_ids=list(range(N_CORES)))
    return _unshard_output(res.results)



# revision 12
# speedup vs baseline: 3.7434x; 1.2566x over previous
"""Trainium2 Bass kernel for nn_DenseEdgeModel (gnn_message_passing).

Reference computation (all 1x1 convs == per-pixel matmuls over channels):
    h    = MLP3(x)                    # 3x (c->c) with ReLU between    [B,C,H,W]
    flat = h as [B*H*W, C]
    xp   = flat[primary_idx]          # [B,PK,C] -> [B,C,PK]
    xc   = flat[compare_idx]          # [B,CK,C] -> [B,C,CK]
    xx   = (xp[..,:,None]-xc[..,None,:])**2          # [B,C,PK,CK]
    g    = relu(W1@xx+b1); g = relu(W2@g+b2)         # over C
    out  = W3@g + b3                  # [B,2,PK,CK]

Sharding (8 cores): data-parallel over batch (4 cores per batch), and the
PK axis split 4-ways within each batch -> each core owns 64 primary
indices of one batch and all 256 compare indices of that batch.

Because the pre-MLP is per-pixel, gather commutes with it:
MLP(x)[idx] == MLP(x[idx]). The host therefore only *slices* (gathers
rows of x for each core's indices and transposes to channel-major).

Device kernel per core:
  stage 1: pre-MLP on the 320 gathered pixel columns [c,320] in fp32r
           (drains split ACT/DVE).  The last layer's weights/bias are
           host-folded by sqrt(SXX) so its bf16 output F is pre-scaled
           for fp8 xx quantization.
  stage 2: per pair of primary columns p ("group"):
    xx   [128,2,512] fp8: channel-chunk 0 via fused ACT Square
           (bias=-xp per primary, FD=256 x2); chunk 1 via DVE
           tensor_scalar subtract (bf16, hits the 4x DVE mode) into a
           bf16 strip, then squared fp8 by one GPSIMD + one DVE
           tensor_tensor (exactly one GPSIMD op per group -- more
           degrades the pipeline).
    l1/l2: ONE fp8e4 DoubleRow matmul per 128-out-chunk (K=256 in a
           single pass at ~2x rate, HW-measured ~230ns for K=256xN=512);
           PSUM holds scale-folded pre-activations; drains
           relu(psum + scaled bias) split ACT/DVE write fp8 (l1) / bf16
           (l2) tiles directly (scales engineered host-side so no
           per-drain rescale op is needed).
    l3   (c->2): bf16 quad-packed into one PSUM bank via tile_position
           col-tiling (bias pre-seeded by a K=1 matmul); one ACT copy +
           4 DMAs drain the quad.  w3 is host-folded by 1/(S1*A2).

Quantization scheme (validated vs reference: rel_err ~1.2e-2 < 2e-2):
  F' = sqrt(SXX)*F;  xx_q = e4m3(SXX*xx);  W1q = e4m3((S1/SXX)*W1)
  psum1 = S1*Z1;  h1_q = e4m3(relu(psum1 + S1*b1)) = e4m3(S1*h1)
  W2q = e4m3(A2*W2);  psum2 = S1*A2*Z2;  h2 = bf16(relu(psum2+S1*A2*b2))
  out = (w3/(S1*A2))@h2 + b3
(e4m3 here is TRN FP8_EXP4: max normal +-240, so scales keep peaks
well under 240.)
"""

import os

import numpy as np

import concourse.bass as bass
import concourse.tile as tile
from concourse import bacc, mybir
from concourse.bass_utils import run_bass_kernel_spmd

# Problem constants (hardcoded per harness contract)
B, C, H, W = 2, 256, 32, 32
PK, CK = 256, 256
N_CORES = 8
CORES_PER_BATCH = N_CORES // B          # 4
P_SHARD = PK // CORES_PER_BATCH         # 64 primary indices per core
NJ = P_SHARD + CK                       # 320 gathered pixel columns per core
PAIR = 2                                # primary columns per stage-2 group
NF = PAIR * CK                          # 512 = stage-2 matmul free dim
QUAD = 4                                # groups sharing one l3 PSUM bank
F32 = mybir.dt.float32
BF16 = mybir.dt.bfloat16
FP8 = mybir.dt.float8e4
MM_DT = mybir.dt.float32r               # stage-1 full-rate fp32 streaming
DR = mybir.MatmulPerfMode.DoubleRow
VARIANT = os.environ.get("KERNEL_VARIANT", "")
AF = mybir.ActivationFunctionType
OP = mybir.AluOpType

# fp8 scale scheme (host-folded; see module docstring)
SXX = 128.0
S1 = 1024.0
A2 = 2048.0


def _build_nc(reps=1):
    nc = bacc.Bacc("TRN2", target_bir_lowering=False, debug=False)

    xgT = nc.dram_tensor("xgT", [C, NJ], F32, kind="ExternalInput")
    pre_wT = nc.dram_tensor("pre_wT", [3, C, C], F32, kind="ExternalInput")
    pre_b = nc.dram_tensor("pre_b", [3, C], F32, kind="ExternalInput")
    # DoubleRow-interleaved fp8 post weights: [oc_chunk, k, k_chunk, m]
    w1q_d = nc.dram_tensor("w1q", [2, 128, 2, 128], FP8, kind="ExternalInput")
    w2q_d = nc.dram_tensor("w2q", [2, 128, 2, 128], FP8, kind="ExternalInput")
    b1s_d = nc.dram_tensor("b1s", [2, 128], F32, kind="ExternalInput")
    b2s_d = nc.dram_tensor("b2s", [2, 128], F32, kind="ExternalInput")
    w3T = nc.dram_tensor("w3T", [C, 2], BF16, kind="ExternalInput")
    # output bias as a [1,128] row: b3[j] at col 32*k+j, used as K=1 lhsT to
    # seed the quad-packed layer-3 PSUM bank (host-built, zero FLOPs)
    b3row_d = nc.dram_tensor("b3row", [1, 128], F32, kind="ExternalInput")
    b3col_d = nc.dram_tensor("b3col", [128, 1], F32, kind="ExternalInput")
    ones_d = nc.dram_tensor("ones_row", [1, NF], F32, kind="ExternalInput")
    oshape = [2, P_SHARD, CK] if reps == 1 else [reps, 2, P_SHARD, CK]
    out = nc.dram_tensor("out", oshape, F32, kind="ExternalOutput")

    with tile.TileContext(nc) as tc:
        for r in range(reps):
            out_r = out.ap() if reps == 1 else out[r]
            _kernel_body(nc, tc, xgT, pre_wT, pre_b, w1q_d, w2q_d, b1s_d,
                         b2s_d, w3T, b3row_d, b3col_d, ones_d, out_r)
    nc.compile()
    return nc


def _kernel_body(nc, tc, xgT, pre_wT, pre_b, w1q_d, w2q_d, b1s_d, b2s_d,
                 w3T, b3row_d, b3col_d, ones_d, out):
    with (
        tc.tile_pool(name="consts", bufs=2) as consts,
        tc.tile_pool(name="feats", bufs=2) as feats,
        tc.tile_pool(name="work",
                     bufs=int(os.environ.get("WORK_BUFS", "3"))) as work,
        tc.tile_pool(name="psum", bufs=1, space="PSUM") as psum,
    ):
        # ---- weights / biases to SBUF (replicated, loaded once) ----
        def col(ap_1d):  # [n] -> [n,1]
            return ap_1d.rearrange("(n one) -> n one", one=1)

        wpre = [[consts.tile([128, C], MM_DT, name=f"wpre_{i}_{cc}")
                 for cc in range(2)] for i in range(3)]
        bpre = [[consts.tile([128, 1], F32, name=f"bpre_{i}_{oc}")
                 for oc in range(2)] for i in range(3)]
        for i in range(3):
            for cc in range(2):
                nc.sync.dma_start(
                    out=wpre[i][cc][:],
                    in_=pre_wT[i, cc * 128:(cc + 1) * 128, :].bitcast(MM_DT))
            for oc in range(2):
                nc.sync.dma_start(out=bpre[i][oc][:],
                                  in_=col(pre_b[i, oc * 128:(oc + 1) * 128]))
        w1q = [consts.tile([128, 2, 128], FP8, name=f"w1q_{oc}")
               for oc in range(2)]
        w2q = [consts.tile([128, 2, 128], FP8, name=f"w2q_{oc}")
               for oc in range(2)]
        b1s = [consts.tile([128, 1], F32, name=f"b1s_{oc}") for oc in range(2)]
        b2s = [consts.tile([128, 1], F32, name=f"b2s_{oc}") for oc in range(2)]
        for oc in range(2):
            nc.sync.dma_start(out=w1q[oc][:], in_=w1q_d[oc])
            nc.sync.dma_start(out=w2q[oc][:], in_=w2q_d[oc])
            nc.sync.dma_start(out=b1s[oc][:], in_=col(b1s_d[oc]))
            nc.sync.dma_start(out=b2s[oc][:], in_=col(b2s_d[oc]))
        w3 = [consts.tile([128, 2], BF16, name=f"w3_{cc}") for cc in range(2)]
        for cc in range(2):
            nc.sync.dma_start(out=w3[cc][:],
                              in_=w3T[cc * 128:(cc + 1) * 128, :])
        b3row = consts.tile([1, 128], MM_DT, name="b3row")
        nc.sync.dma_start(out=b3row[:], in_=b3row_d.ap().bitcast(MM_DT))
        b3col = consts.tile([128, 1], F32, name="b3col")
        nc.sync.dma_start(out=b3col[:], in_=b3col_d.ap())
        ones_row = consts.tile([1, NF], MM_DT, name="ones_row")
        nc.sync.dma_start(out=ones_row[:], in_=ones_d.ap().bitcast(MM_DT))

        # ---- stage 1: pre-MLP over the 320 gathered pixel columns ----
        cur = [feats.tile([128, NJ], MM_DT, name=f"xg_{cc}") for cc in range(2)]
        for cc in range(2):
            nc.sync.dma_start(out=cur[cc][:],
                              in_=xgT[cc * 128:(cc + 1) * 128, :].bitcast(MM_DT))
        for i in range(3):
            nxt = [feats.tile([128, NJ], MM_DT if i < 2 else BF16,
                              name=f"feat{i}_{oc}") for oc in range(2)]
            for oc in range(2):
                ps = psum.tile([128, NJ], F32, name=f"ps_s1_{i}_{oc}",
                               tag="ps_a", bufs=3)
                nc.tensor.matmul(
                    out=ps[:],
                    lhsT=wpre[i][0][:, oc * 128:(oc + 1) * 128],
                    rhs=cur[0][:], start=True, stop=False)
                nc.tensor.matmul(
                    out=ps[:],
                    lhsT=wpre[i][1][:, oc * 128:(oc + 1) * 128],
                    rhs=cur[1][:], start=False, stop=True)
                if oc == 0:
                    nc.scalar.activation(
                        out=nxt[oc][:], in_=ps[:],
                        func=AF.Relu if i < 2 else AF.Identity,
                        bias=bpre[i][oc][:, 0:1], scale=1.0)
                elif i < 2:
                    nc.vector.tensor_scalar(
                        out=nxt[oc][:], in0=ps[:],
                        scalar1=bpre[i][oc][:, 0:1], scalar2=0.0,
                        op0=OP.add, op1=OP.max)
                else:
                    nc.vector.tensor_scalar(
                        out=nxt[oc][:], in0=ps[:],
                        scalar1=bpre[i][oc][:, 0:1], scalar2=None,
                        op0=OP.add)
            cur = nxt
        F = cur  # [128, NJ] bf16 x2 chunks; 0:P_SHARD primary, rest compare
        # F is pre-scaled by sqrt(SXX) via host-folded pre_w[2]/pre_b[2]
        # fp32 copy of the primary columns (tensor_scalar scalars must be f32)
        Fp = [feats.tile([128, P_SHARD], F32, name=f"Fp_{cc}")
              for cc in range(2)]
        for cc in range(2):
            nc.vector.tensor_copy(out=Fp[cc][:], in_=F[cc][:, 0:P_SHARD])

        # ---- stage 2: software-pipelined with a 4-deep skew ----
        NG = P_SHARD // PAIR
        if VARIANT == "half":
            NG = NG // 2
        d_t, xx_t, h1_t, h2_t, ps3_t = {}, {}, {}, {}, {}

        # engine maps (tuned on HW via interleaved A/B): 4 squares (s,cc) and
        # 4 drains (l1o0 l1o1 l2o0 l2o1).  GPSIMD cannot touch PSUM, so
        # drains are act/dve only; >1 GPSIMD op per group measurably degrades
        # the pipeline (strict-FIFO Q7 dispatch), so at most one is used.
        # Override via ASSIGN="sq:act,gp,dve,gp;dr:act,dve,act,dve;cp:dve"
        SQ_ENG = ("gpdve",)
        DRAIN_ENG = ("act", "dve", "act", "dve")
        CP_ENG = "act"
        for part in os.environ.get("ASSIGN", "").split(";"):
            if part.startswith("sq:"):
                SQ_ENG = tuple(part[3:].split(","))
            elif part.startswith("dr:"):
                DRAIN_ENG = tuple(part[3:].split(","))
            elif part.startswith("cp:"):
                CP_ENG = part[3:]
        L3_DMA = False  # bass dma_start cannot read PSUM
        XX_MODE = os.environ.get("XX_MODE", "mix")

        def stage_xx(g):
            if XX_MODE == "act1":
                # one fused ACT op per slice: (xp - xc)^2 -> fp8
                xx = work.tile([128, 2, NF], FP8, name="xx", tag="xx")
                for s in range(PAIR):
                    p = g * PAIR + s
                    for cc in range(2):
                        nc.scalar.activation(
                            out=xx[:, cc, s * CK:(s + 1) * CK],
                            in_=F[cc][:, P_SHARD:NJ],
                            func=AF.Square,
                            bias=Fp[cc][:, p:p + 1], scale=-1.0)
                xx_t[g] = xx
                return
            if XX_MODE == "mix":
                # chunk 0: fused ACT Square per primary; chunk 1: DVE 4x subs
                # into a bf16 d-strip, then one merged FD-512 square.
                xx = work.tile([128, 2, NF], FP8, name="xx", tag="xx")
                d = work.tile([128, NF], BF16, name="d", tag="d")
                for s in range(PAIR):
                    p = g * PAIR + s
                    nc.scalar.activation(
                        out=xx[:, 0, s * CK:(s + 1) * CK],
                        in_=F[0][:, P_SHARD:NJ],
                        func=AF.Square,
                        bias=Fp[0][:, p:p + 1], scale=-1.0)
                    nc.vector.tensor_scalar(
                        out=d[:, s * CK:(s + 1) * CK],
                        in0=F[1][:, P_SHARD:NJ],
                        scalar1=Fp[1][:, p:p + 1], scalar2=None,
                        op0=OP.subtract)
                if SQ_ENG[0] == "act":
                    nc.scalar.activation(out=xx[:, 1, :], in_=d[:],
                                         func=AF.Square)
                elif SQ_ENG[0] == "gpdve":
                    nc.gpsimd.tensor_tensor(out=xx[:, 1, 0:CK],
                                            in0=d[:, 0:CK], in1=d[:, 0:CK],
                                            op=OP.mult)
                    nc.vector.tensor_tensor(out=xx[:, 1, CK:NF],
                                            in0=d[:, CK:NF], in1=d[:, CK:NF],
                                            op=OP.mult)
                elif SQ_ENG[0] == "gp":
                    # two FD-256 GP ops (cheaper than one FD-512 on the Q7s)
                    for s in range(PAIR):
                        sl = slice(s * CK, (s + 1) * CK)
                        nc.gpsimd.tensor_tensor(out=xx[:, 1, sl],
                                                in0=d[:, sl], in1=d[:, sl],
                                                op=OP.mult)
                else:
                    nc.vector.tensor_tensor(out=xx[:, 1, :], in0=d[:],
                                            in1=d[:], op=OP.mult)
                xx_t[g] = xx
                return
            # d[:, cc, s*CK:(s+1)*CK] = F[cc][:, cmp] - F[cc][:, p_s]  (bf16,
            # DVE tensor_scalar hits the 4x path), then squared into fp8.
            d = work.tile([128, 2, NF], BF16, name="d", tag="d")
            for s in range(PAIR):
                p = g * PAIR + s
                for cc in range(2):
                    nc.vector.tensor_scalar(
                        out=d[:, cc, s * CK:(s + 1) * CK],
                        in0=F[cc][:, P_SHARD:NJ],
                        scalar1=Fp[cc][:, p:p + 1], scalar2=None,
                        op0=OP.subtract)
            d_t[g] = d

        def stage_sq(g):
            if XX_MODE in ("act1", "mix"):
                return
            d = d_t.pop(g)
            xx = work.tile([128, 2, NF], FP8, name="xx", tag="xx")
            for s in range(PAIR):
                for cc in range(2):
                    src = d[:, cc, s * CK:(s + 1) * CK]
                    dst = xx[:, cc, s * CK:(s + 1) * CK]
                    eng = SQ_ENG[2 * s + cc]
                    if eng == "act":
                        nc.scalar.activation(out=dst, in_=src, func=AF.Square)
                    elif eng == "dve":
                        nc.vector.tensor_tensor(out=dst, in0=src, in1=src,
                                                op=OP.mult)
                    else:
                        nc.gpsimd.tensor_tensor(out=dst, in0=src, in1=src,
                                                op=OP.mult)
            xx_t[g] = xx

        def drain(which, out_ap, ps, bias):
            # relu(psum + bias) -> out (fp8/bf16); scales are pre-folded
            if which == "act":
                nc.scalar.activation(out=out_ap, in_=ps[:], func=AF.Relu,
                                     bias=bias[:, 0:1], scale=1.0)
            else:
                nc.vector.tensor_scalar(
                    out=out_ap, in0=ps[:], scalar1=bias[:, 0:1], scalar2=0.0,
                    op0=OP.add, op1=OP.max)

        def stage_l1(g):
            xx = xx_t.pop(g)
            h1 = work.tile([128, 2, NF], FP8, name="h1", tag="h1")
            for oc in range(2):
                ps = psum.tile([128, NF], F32, name=f"ps_l1_{oc}",
                               tag="ps_a", bufs=3)
                nc.tensor.matmul(out=ps[:], lhsT=w1q[oc][:], rhs=xx[:],
                                 perf_mode=DR, start=True, stop=True)
                drain(DRAIN_ENG[oc], h1[:, oc, :], ps, b1s[oc])
            h1_t[g] = h1

        def stage_l2(g):
            h1 = h1_t.pop(g)
            # bufs=6: up to 4 quad members live awaiting the l3 burst, plus
            # pipeline slack so the l2 drain never waits on the burst's reads
            h2 = work.tile([128, 2, NF], BF16, name="h2", tag="h2", bufs=6)
            for oc in range(2):
                ps = psum.tile([128, NF], F32, name=f"ps_l2_{oc}",
                               tag="ps_b", bufs=3)
                nc.tensor.matmul(out=ps[:], lhsT=w2q[oc][:], rhs=h1[:],
                                 perf_mode=DR, start=True, stop=True)
                drain(DRAIN_ENG[2 + oc], h2[:, oc, :], ps, b2s[oc])
            h2_t[g] = h2

        def stage_l3(q):
            # layer 3 (c->2) for a whole quad of 4 groups, emitted as ONE
            # back-to-back PE burst: bias seed (K=1, defines every row), then
            # the 8 col-tiled matmuls ordered cc-major so the 4 distinct
            # col-groups (tile_position=(0,32k)) stream CONCURRENTLY on
            # disjoint 32-col strips of the array (per-subarray overlap,
            # ~2 serialized rounds of 512 cols instead of 8).  Math and
            # start/stop flags are identical to the per-group form.
            ps3 = psum.tile([128, NF], F32, name="ps3", tag="ps3", bufs=2)
            if q < 2:
                # define the full bank ONCE per physical buffer so the quad
                # drain's full-128-partition copy only ever reads finite
                # memory (rows outside 32k..32k+1 are never matmul targets);
                # later quads overwrite their rows via start=True below
                nc.tensor.matmul(out=ps3[:], lhsT=b3row[:], rhs=ones_row[:],
                                 start=True, stop=True)
            hq = [h2_t.pop(q * QUAD + k) for k in range(QUAD)]
            for cc in range(2):
                for k in range(QUAD):
                    nc.tensor.matmul(
                        out=ps3[32 * k:32 * k + 2, :], lhsT=w3[cc][:],
                        rhs=hq[k][:, cc, :], tile_position=(0, 32 * k),
                        start=(cc == 0), stop=True, skip_group_check=True)
            if True:
                qb = q * QUAD * PAIR
                if L3_DMA:
                    for kk in range(QUAD):
                        nc.sync.dma_start(
                            out=out[:, qb + kk * PAIR:qb + (kk + 1) * PAIR, :],
                            in_=ps3[32 * kk:32 * kk + 2, :].rearrange(
                                "j (s q) -> j s q", s=PAIR))
                else:
                    ob = work.tile([128, NF], F32, name="ob", tag="ob", bufs=2)
                    # b3 rides on the drain copy (bias is per-partition, b3col
                    # holds b3[j] at rows 32k+j) instead of a seed matmul
                    if CP_ENG == "act":
                        nc.scalar.activation(out=ob[:], in_=ps3[:],
                                             func=AF.Identity,
                                             bias=b3col[:, 0:1], scale=1.0)
                    else:
                        nc.vector.tensor_scalar(
                            out=ob[:], in0=ps3[:], scalar1=b3col[:, 0:1],
                            scalar2=None, op0=OP.add)
                    for kk in range(QUAD):
                        nc.sync.dma_start(
                            out=out[:, qb + kk * PAIR:qb + (kk + 1) * PAIR, :],
                            in_=ob[32 * kk:32 * kk + 2, :].rearrange(
                                "j (s q) -> j s q", s=PAIR))

        # Emit the independent producer stages (xx subs, squares) BEFORE the
        # PSUM drains each iteration: ACT/DVE are strict-FIFO, so a drain
        # waiting on PE at queue head would head-of-line-block the subs.
        for i in range(NG + 4):
            if i < NG:
                stage_xx(i)
            if 1 <= i < NG + 1:
                stage_sq(i - 1)
            if 2 <= i < NG + 2:
                stage_l1(i - 2)
            if 3 <= i < NG + 3:
                stage_l2(i - 3)
            # l3 fires once per quad, after the whole quad's h2 is emitted:
            # quad q's last member (4q+3) finishes stage_l2 at i = 4q+6
            if i >= 7 and (i - 7) % QUAD == 0:
                stage_l3((i - 7) // QUAD)


_NC_CACHE = {}


def _get_nc():
    if "nc" not in _NC_CACHE:
        _NC_CACHE["nc"] = _build_nc()
    return _NC_CACHE["nc"]


def _shard_inputs(x, primary_indices, compare_indices, pre_w, pre_b,
                  post_w, post_b, post_out_w, post_out_b):
    """Host-side sharding: per-core index slicing + row gather of x (pure
    data movement -- the pre-MLP commutes with the gather), weight
    transposes, and fp8 scale folding. Returns the 8 per-core input maps."""
    import ml_dtypes
    E4M3 = ml_dtypes.float8_e4m3
    x = np.asarray(x, dtype=np.float32)
    x_rows = np.ascontiguousarray(x.transpose(0, 2, 3, 1)).reshape(B * H * W, C)
    pre_wT = np.asarray(pre_w, dtype=np.float32).transpose(0, 2, 1).copy()
    pre_b = np.asarray(pre_b, dtype=np.float32).copy()
    # fold sqrt(SXX) into the last pre layer so F is pre-scaled for xx fp8
    rt = np.float32(np.sqrt(SXX))
    pre_wT[2] *= rt
    pre_b[2] *= rt

    post_w = np.asarray(post_w, dtype=np.float32)
    post_b = np.asarray(post_b, dtype=np.float32)
    # DoubleRow-interleaved fp8 weights [oc_chunk, k, k_chunk, m]:
    # arr[o,c] -> o=(j,m), c=(i,k) -> [j,k,i,m]
    def dr_pack(wmat, scale):
        a = (wmat * scale).reshape(2, 128, 2, 128)   # [j, m, i, k]
        return np.ascontiguousarray(a.transpose(0, 3, 2, 1)).astype(E4M3)

    w1q = dr_pack(post_w[0], S1 / SXX)
    w2q = dr_pack(post_w[1], A2)
    b1s = np.ascontiguousarray((post_b[0] * S1).reshape(2, 128))
    b2s = np.ascontiguousarray((post_b[1] * (S1 * A2)).reshape(2, 128))

    w3T = np.ascontiguousarray(
        (np.asarray(post_out_w, dtype=np.float32) / (S1 * A2)).T
    ).astype(ml_dtypes.bfloat16)
    b3 = np.asarray(post_out_b, dtype=np.float32)
    b3row = np.zeros((1, 128), dtype=np.float32)
    for k in range(QUAD):
        b3row[0, 32 * k:32 * k + 2] = b3
    b3col = b3row.reshape(128, 1).copy()
    primary_indices = np.asarray(primary_indices)
    compare_indices = np.asarray(compare_indices)

    in_maps = []
    for core in range(N_CORES):
        b = core // CORES_PER_BATCH
        ps = (core % CORES_PER_BATCH) * P_SHARD
        rows = np.concatenate([
            primary_indices[b, ps:ps + P_SHARD].astype(np.int64),
            compare_indices[b].astype(np.int64),
        ])
        xg_T = np.ascontiguousarray(x_rows[rows].T)  # [C, NJ]
        in_maps.append({
            "xgT": xg_T,
            "pre_wT": pre_wT,
            "pre_b": pre_b,
            "w1q": w1q,
            "w2q": w2q,
            "b1s": b1s,
            "b2s": b2s,
            "w3T": w3T,
            "b3row": b3row,
            "b3col": b3col,
            "ones_row": np.ones((1, NF), dtype=np.float32),
        })
    return in_maps


def _unshard_output(results):
    out = np.empty((B, 2, PK, CK), dtype=np.float32)
    for core in range(N_CORES):
        b = core // CORES_PER_BATCH
        ps = (core % CORES_PER_BATCH) * P_SHARD
        out[b, :, ps:ps + P_SHARD, :] = results[core]["out"]
    return out


def kernel(x, primary_indices, compare_indices, pre_w, pre_b,
           post_w, post_b, post_out_w, post_out_b):
    in_maps = _shard_inputs(x, primary_indices, compare_indices, pre_w, pre_b,
                            post_w, post_b, post_out_w, post_out_b)
    nc = _get_nc()
    res = run_bass_kernel_spmd(nc, in_maps, core_ids=list(range(N_CORES)))
    return _unshard_output(res.results)

